# revision 1
# baseline (speedup 1.0000x reference)
"""Trainium2 Bass kernel for nn_Conv2d_72430328481302.

Conv2d: input (16,128,56,56) f32, weight (128,128,3,3), bias (128),
stride 1, pad 1, dilation 1 -> output (16,128,56,56).

Strategy:
  - Data-parallel over batch: 2 images per core across 8 cores, weight
    replicated.
  - The host pre-pads each image to a [Cin=128, 58*58] zero-framed
    plane (bf16) so the per-image input DMA is one fully contiguous
    transfer per chunk (line-rate, no tiny descriptors).
  - A 3x3 stride-1 conv is then 9 accumulating bf16 matmuls per
    8-output-row slab: for tap (kh,kw) the moving operand is the
    padded plane at offset (s*8+kh)*58+kw viewed as [8 rows x 56] (row
    stride 58), so the psum output is the dense 448-column slab.
    bf16 runs the PE at 1 cycle/row (f32r is 2 cycles/row on HW) and
    f32 PSUM accumulation keeps rel err ~2e-3, far under the 2e-2 gate.
  - The chip's DVFS clock reaches full speed only several us into the
    NEFF (engines AND DMA run slow before that, and PE idle pauses the
    ramp), so dummy warmup matmuls on scratch SBUF keep the PE busy
    from preamble-end: a tiny 16-col scratch first (its memset is ~10x
    cheaper, so the PE starts ~1us earlier) then 448-col ones, sized
    so real data arrives as they finish. The kh=0 weight taps lead on
    the scalar ring so input packets can't starve them on the shared
    physical DMA engines.
  - Input is issued up-front on the sync ring in 5 chunks per image so
    no single late chunk can stall a matmul group when the DMA clock
    is in its slow regime.
  - PSUM is evacuated by the scalar engine with the bias add fused;
    mid-kernel outputs leave on the scalar ring. Image 1 ends with
    single-slab groups whose halves are evacuated on scalar+vector in
    parallel and stored with the two rings' assignments crossed, so
    each ring carries one early and one late half and the final drain
    is halved.
"""

import os
import sys

for _p in ("/opt/trn_rl_repo",):
    if os.path.isdir(_p) and _p not in sys.path:
        sys.path.insert(0, _p)

import ml_dtypes
import numpy as np

import concourse.bass as bass
import concourse.tile as tile
from concourse import bacc, mybir
from concourse.bass_utils import run_bass_kernel_spmd

N_CORES = 8
N_IMGS = 16
IPC = N_IMGS // N_CORES  # images per core
CIN = 128
COUT = 128
H = W = 56
WP = 58  # padded width (1 col each side)
HP = 58  # padded height (1 row each side)
FLATP = HP * WP  # 3364
PAD_ALLOC = 3376  # pad per-partition row to a 32B multiple (bf16)
RS = 8  # output rows per slab
NSLAB = H // RS  # 7
SLAB_N = RS * W  # 448 psum columns per slab (dense)
F32 = mybir.dt.float32
BF16 = mybir.dt.bfloat16

_CACHE = {}


def _build_nc():
    nc = bacc.Bacc(
        "TRN2",
        target_bir_lowering=False,
        debug=False,
        num_devices=N_CORES,
    )
    x = nc.dram_tensor("x", [IPC, CIN, PAD_ALLOC], BF16, kind="ExternalInput")
    wt = nc.dram_tensor("wt", [CIN, 9, COUT], BF16, kind="ExternalInput")
    bvec = nc.dram_tensor("bvec", [COUT, 1], F32, kind="ExternalInput")
    # output travels as bf16 (halves store wire time; host casts back to
    # f32 — adds ~0.2% RMS, far under the 2e-2 gate)
    y = nc.dram_tensor("y", [IPC, COUT, H * W], BF16, kind="ExternalOutput")

    # raw (non-pool) scratch for PE warmup: outside the tile framework
    # there is no written-before-read requirement, so the first warmup
    # matmul issues right after the PE preamble with no memset gate;
    # the garbage values never leave PSUM (start=True resets each time)
    scrw = nc.alloc_sbuf_tensor("scrw", [CIN, 576], BF16)

    with tile.TileContext(nc) as tc:
        with (
            tc.tile_pool(name="const", bufs=1) as cpool,
            tc.tile_pool(name="xin", bufs=1) as xpool,
            tc.tile_pool(name="yout", bufs=1) as ypool,
            tc.tile_pool(name="psum", bufs=8, space="PSUM") as pspool,
        ):
            # PE warmup: matmuls on raw scratch SBUF (no deps at all, so
            # they run right after the engine preamble and keep the PE
            # busy — ramping the DVFS clock — while the first
            # input/weight DMAs land)
            wps = pspool.tile([COUT, SLAB_N], F32, name="wps", tag="ps")
            for wi in range(7):
                nc.tensor.matmul(
                    wps[:],
                    scrw.ap()[:, 0:128],
                    scrw.ap()[:, 128 : 128 + SLAB_N],
                    start=True, stop=True,
                )
            for wi in range(4):
                nc.tensor.matmul(
                    wps[:, 0:128],
                    scrw.ap()[:, 0:128],
                    scrw.ap()[:, 128:256],
                    start=True, stop=True,
                )

            wt_sb = cpool.tile([CIN, 9, COUT], BF16, name="wt_sb", tag="wt_sb")
            bias_sb = cpool.tile([COUT, 1], F32, name="bias_sb", tag="bias_sb")
            P = {}
            for i in range(IPC):
                P[i] = xpool.tile(
                    [CIN, PAD_ALLOC], BF16, name=f"P{i}", tag=f"P{i}"
                )
            # kh=0 taps lead on the scalar ring: they gate the first
            # matmul and must not queue behind input packets on the
            # shared physical DMA engines; input data must NOT ride the
            # scalar ring (it lands ~2.5us late behind sync traffic)
            # three weight pieces so the kh=1 taps don't wait for the kh=2
            # ones to cross the wire when the DMA clock is slow
            nc.scalar.dma_start(wt_sb[:, 0:3, :], wt.ap()[:, 0:3, :])
            nc.scalar.dma_start(wt_sb[:, 3:6, :], wt.ap()[:, 3:6, :])
            nc.scalar.dma_start(wt_sb[:, 6:9, :], wt.ap()[:, 6:9, :])
            nc.scalar.dma_start(bias_sb[:], bvec.ap()[:])

            # all input chunks issue up-front on the sync ring; finer
            # chunks bound the cost of a late piece when the DMA clock
            # is in its slow regime (no single large chunk gates a
            # whole matmul group)
            CHUNKS = {
                0: [(0, 12), (12, 18), (18, 30), (30, 42), (42, 50), (50, HP)],
                1: [(0, 18), (18, 30), (30, 42), (42, 50), (50, HP)],
            }
            for i in range(IPC):
                for r0, r1 in CHUNKS[i]:
                    e0, e1 = r0 * WP, (r1 * WP if r1 < HP else PAD_ALLOC)
                    nc.sync.dma_start(P[i][:, e0:e1], x.ap()[i, :, e0:e1])

            # image 1 ends with single-slab groups so slab 5's halves
            # drain while slab 6 is still computing
            GROUPS = {
                0: [(0, 1), (2, 3, 4), (5, 6)],
                1: [(0, 1), (2, 3, 4), (5,), (6,)],
            }
            TAIL = {5, 6}  # image-1 slabs with half-split evac/store
            HALF = SLAB_N // 2  # tail evac/store split unit
            for i in range(IPC):
                out_sb = ypool.tile(
                    [COUT, H * W], BF16, name=f"out{i}", tag=f"out{i}"
                )
                for grp in GROUPS[i]:
                    pss = {
                        s: pspool.tile(
                            [COUT, SLAB_N], F32, name=f"ps_{i}_{s}", tag="ps"
                        )
                        for s in grp
                    }
                    # tap-outer within the group: consecutive matmuls share
                    # the stationary weights
                    for t, (kh, kw) in enumerate(
                        (a, b) for a in range(3) for b in range(3)
                    ):
                        for s in grp:
                            start = (s * RS + kh) * WP + kw
                            rhs = P[i][:, start : start + RS * WP].rearrange(
                                "c (r k) -> c r k", k=WP
                            )[:, :, 0:W]
                            nc.tensor.matmul(
                                pss[s][:],
                                wt_sb[:, kh * 3 + kw, :],
                                rhs,
                                start=(t == 0),
                                stop=(t == 8),
                            )
                    # evacuate + fused bias add, then store the group.
                    # The final group of the final image splits each slab's
                    # evac across the scalar and vector engines and its
                    # stores across the sync and scalar rings, shortening
                    # the kernel's drain tail.
                    tail_grp = i == IPC - 1 and all(s in TAIL for s in grp)
                    if tail_grp:
                        # end slabs, halves on separate engines/rings.
                        # Slab 5 (early): evac scalar+vector, both stores
                        # on the idle sync queue — a store on the scalar
                        # queue would be scheduled behind slab 6's evac
                        # wait and drain late. Slab 6 (last): evac
                        # gpsimd+vector (no scalar ACT on the critical
                        # tail), stores split sync/scalar.
                        (s,) = grp
                        b = s * SLAB_N
                        d0 = out_sb[:, b : b + HALF]
                        d1 = out_sb[:, b + HALF : b + SLAB_N]
                        nc.vector.tensor_scalar_add(
                            d1, pss[s][:, HALF:SLAB_N], bias_sb[:, :]
                        )
                        nc.scalar.activation(
                            d0,
                            pss[s][:, 0:HALF],
                            mybir.ActivationFunctionType.Identity,
                            bias=bias_sb[:, :],
                        )
                        if s == 5:
                            nc.sync.dma_start(
                                y.ap()[i, :, b : b + HALF], d0,
                                single_packet=True,
                            )
                            nc.sync.dma_start(
                                y.ap()[i, :, b + HALF : b + SLAB_N], d1,
                                single_packet=True,
                            )
                        else:
                            nc.sync.dma_start(
                                y.ap()[i, :, b : b + HALF], d0,
                                single_packet=True,
                            )
                            nc.scalar.dma_start(
                                y.ap()[i, :, b + HALF : b + SLAB_N], d1,
                                single_packet=True,
                            )
                    else:
                        for s in grp:
                            dst = out_sb[:, s * SLAB_N : (s + 1) * SLAB_N]
                            nc.scalar.activation(
                                dst,
                                pss[s][:],
                                mybir.ActivationFunctionType.Identity,
                                bias=bias_sb[:, :],
                            )
                        c0, c1 = grp[0] * SLAB_N, (grp[-1] + 1) * SLAB_N
                        nc.scalar.dma_start(
                            y.ap()[i, :, c0:c1], out_sb[:, c0:c1]
                        )

    nc.compile()
    return nc


def _get_nc():
    if "nc" not in _CACHE:
        _CACHE["nc"] = _build_nc()
    return _CACHE["nc"]


def _make_in_maps(input, weight, bias):
    input = np.asarray(input)
    weight = np.asarray(weight)
    bias = np.asarray(bias)
    # pad every image into the [IPC, CIN, 58*58 (+pad)] zero-framed plane
    padded = np.zeros((N_IMGS, CIN, PAD_ALLOC), dtype=ml_dtypes.bfloat16)
    pv = padded[:, :, :FLATP].reshape(N_IMGS, CIN, HP, WP)
    pv[:, :, 1 : H + 1, 1 : W + 1] = input
    # weight (Cout,Cin,3,3) -> lhsT layout (Cin, kh*3+kw, Cout)
    wt_host = np.ascontiguousarray(
        np.transpose(weight, (1, 2, 3, 0)).reshape(CIN, 9, COUT)
    ).astype(ml_dtypes.bfloat16)
    b_host = np.ascontiguousarray(bias.reshape(COUT, 1), dtype=np.float32)
    return [
        {
            "x": padded[c * IPC : (c + 1) * IPC],
            "wt": wt_host,
            "bvec": b_host,
        }
        for c in range(N_CORES)
    ]


def run(input, weight, bias, trace=False, tmpdir=None):
    """Run the SPMD kernel; returns (output, BassKernelResults)."""
    nc = _get_nc()
    in_maps = _make_in_maps(input, weight, bias)
    res = run_bass_kernel_spmd(
        nc, in_maps, list(range(N_CORES)), trace=trace, tmpdir=tmpdir
    )
    out = np.concatenate(
        [np.asarray(res.results[c]["y"]) for c in range(N_CORES)], axis=0
    ).astype(np.float32)
    return out.reshape(N_IMGS, COUT, H, W).astype(np.float32), res


def kernel(input, weight, bias):
    out, _ = run(input, weight, bias, trace=False)
    return out

